# revision 32
# baseline (speedup 1.0000x reference)
"""Trainium2 Bass kernel for CustomMultiHeadAttention (B=4, S=1024, D=1024, H=16, Dh=64).

Sharding: 8 cores = (batch b in 0..3) x (parity par in 0..1).
Core (b, par) computes output rows {s : s % 2 == par} of batch b,
grouped into 4 "vblocks" of 128 rows (vblock i' = seq 256*i' + 2*c + par).
K/V are computed for the full sequence on every core (from the full x[b]).
The program is identical on all cores; per-core differences are input data.

Pipeline (all transposed-layout, PE-centric):
  QT = rope(Wq^T x^T), KT = rope(Wk^T x^T)  - rope via permutation-matmul + DVE
  scT[kv,q] = KT_h^T QT_h (2 heads row-packed), exp on ScalarE (scale=1/8),
  causal mask = f16 0/1 multiply on the diagonal 128 cols,
  ctxT/denoms accumulate via lhsT=[V|1], normalize via reciprocal_approx_fast
  + PE broadcast, out = ctxT^T Wo.
"""

import threading

import numpy as np

B, S, D, H, Dh = 4, 1024, 1024, 16, 64
P = 128
N_CORES = 8
NT = D // P  # 8 tiles along d/dout/seq
# q col c <-> seq 2c+par (plain order); for kv-block j the active q
# cols are the suffix [64j, 512) and the mask covers its first 64 cols
NJ = [512 - 64 * j for j in range(8)]
VS = 65  # V slot width: [V(64) | ones(1)] per head

_cache = {}
_lock = threading.Lock()


def _build_program(taps=False):
    import concourse.bass as bass  # noqa: F401
    import concourse.mybir as mybir
    import concourse.tile as tile
    from concourse import bacc

    dt = mybir.dt
    f16, f32 = dt.float16, dt.float32
    AF = mybir.ActivationFunctionType

    nc = bacc.Bacc("TRN2", target_bir_lowering=False, debug=False,
                   num_devices=N_CORES)

    def ein(name, shape):
        return nc.dram_tensor(name, shape, f16, kind="ExternalInput").ap()

    xt_sh = ein("xt_sh", [P, NT, S])     # x[b]^T, host-transposed
    xqt_sh = ein("xqt_sh", [P, NT, 512])  # xq^T, host-transposed
    w_ext = {n: ein(n, [D, D]) for n in ("wq", "wk", "wv", "wo")}
    bqt_e = nc.dram_tensor("bqt", [P, NT], f32, kind="ExternalInput").ap()
    bkt_e = nc.dram_tensor("bkt", [P, NT], f32, kind="ExternalInput").ap()
    bvb_e = ein("bvb", [P, D])           # bv broadcast across partitions
    bob_e = ein("bob", [P, D])           # bo broadcast across partitions
    cosq_e = ein("cosq", [P, 512])
    sinq_e = ein("sinq", [P, 512])
    cosk_e = ein("cosk", [P, S])
    sink_e = ein("sink", [P, S])
    mj_e = ein("mj", [P, 64])
    p128_e = ein("p128", [P, P])
    y_sh = nc.dram_tensor("y_sh", [512, D], f16, kind="ExternalOutput").ap()
    tap_ext = {}
    if taps:
        for tn, shape in (("qt", [P, NT, 512]), ("kt", [P, NT, S]),
                          ("v1", [P, NT, H * VS]), ("cn", [P, NT, 512])):
            tap_ext[tn] = nc.dram_tensor("dbg_" + tn, shape, f16,
                                         kind="ExternalOutput").ap()

    with tile.TileContext(nc) as tc:
        from contextlib import ExitStack
        with ExitStack() as ctx:
            big = ctx.enter_context(tc.tile_pool(name="big", bufs=1))

            xT = big.tile([P, NT, S], f16, tag="xT")        # x[b]^T  [din, s]
            xqT = big.tile([P, NT, 512], f16, tag="xqT")    # xq^T    [din, q]
            w_sb = {n: big.tile([P, NT, D], f16, tag=n, name=n + "_sb")
                    for n in w_ext}
            bqt = big.tile([P, NT], f32, tag="bqt")
            bkt = big.tile([P, NT], f32, tag="bkt")
            bvb = big.tile([P, D], f16, tag="bvb")
            bob = big.tile([P, D], f16, tag="bob")
            qt = big.tile([P, NT, 512], f16, tag="qt")      # rope'd Q^T
            kt = big.tile([P, NT, S], f16, tag="kt")        # rope'd K^T
            v1 = big.tile([P, NT, H * VS], f16, tag="v1")   # [V|1] slots
            cn = big.tile([P, NT, 512], f16, tag="cn")      # normalized ctx^T
            cosq = big.tile([P, 512], f16, tag="cosq")
            sinq = big.tile([P, 512], f16, tag="sinq")
            cosk = big.tile([P, S], f16, tag="cosk")
            sink = big.tile([P, S], f16, tag="sink")
            mj = big.tile([P, 64], f16, tag="mj")
            p128 = big.tile([P, P], f16, tag="p128")

            # ---- input DMAs ----
            # Coarse DMAs: one dma_start per tensor chunk (issue overhead
            # is ~0.6us each, so fewer+bigger wins). Order on the sync
            # queue matches first-use order of the projection chains.
            def wslice(name, c0, c1):
                src = w_ext[name].rearrange("(k p) d -> p k d", p=P)
                nc.sync.dma_start(w_sb[name][:, :, c0:c1],
                                  src[:, :, c0:c1])

            nc.sync.dma_start(xqT[:], xqt_sh[:])
            wslice("wq", 0, 512)
            for t, e in ((p128, p128_e), (cosq, cosq_e),
                         (sinq, sinq_e), (bqt, bqt_e)):
                nc.sync.dma_start(t[:], e[:])
            nc.sync.dma_start(xT[:, :, 0:512], xt_sh[:, :, 0:512])
            wslice("wk", 0, 512)
            for t, e in ((cosk, cosk_e), (sink, sink_e), (bkt, bkt_e)):
                nc.sync.dma_start(t[:], e[:])
            nc.sync.dma_start(xT[:, :, 512:1024], xt_sh[:, :, 512:1024])
            wslice("wq", 512, 1024)
            wslice("wk", 512, 1024)
            # bulk weights on the gpsimd queue, in parallel
            nc.gpsimd.dma_start(
                w_sb["wv"][:],
                w_ext["wv"].rearrange("(k p) d -> p k d", p=P))
            for t, e in ((bvb, bvb_e), (mj, mj_e)):
                nc.gpsimd.dma_start(t[:], e[:])
            nc.gpsimd.dma_start(
                w_sb["wo"][:],
                w_ext["wo"].rearrange("(k p) d -> p k d", p=P))
            nc.gpsimd.dma_start(bob[:], bob_e[:])
            # ones columns of the V slots (col 64 of each 65-wide slot)
            v1r = v1.rearrange("p t (h c) -> p t h c", c=VS)
            for t in range(NT):
                nc.any.memset(v1r[:, t, :, 64:65], 1.0)

            # ---- projections + rope + attention, interleaved ----
            # Projections use a single rotating PSUM tag (3 banks); the
            # perm-matmul for rope unit u-1 is emitted after unit u's chain
            # so the PE never waits on the ScalarE psum->sbuf evacuation.
            # Attention head-pair p is emitted right after the t=p+1
            # projections, so its ScalarE exp work overlaps the remaining
            # projection matmuls instead of serializing behind them.
            # PSUM banks: pp 3 + scp 3 + cxp 2 = 8.
            with tc.tile_pool(name="pp", bufs=3, space="PSUM") as pp, \
                 tc.tile_pool(name="sc", bufs=4) as sc, \
                 tc.tile_pool(name="scp", bufs=3, space="PSUM") as scp, \
                 tc.tile_pool(name="cxp", bufs=1, space="PSUM") as cxp, \
                 tc.tile_pool(name="ep", bufs=3) as ep, \
                 tc.tile_pool(name="npl", bufs=2) as npl:
                pending = []

                def flush_one():
                    dst, raw, cos_ap, sin_ap = pending.pop(0)
                    pq = pp.tile([P, 512], f32, tag="ps", name="pq")
                    nc.tensor.matmul(pq[:], p128[:], raw[:],
                                     start=True, stop=True)
                    t1 = sc.tile([P, 512], f16, tag="t1", name="t1")
                    nc.vector.tensor_mul(t1[:], raw[:], cos_ap)
                    t2 = sc.tile([P, 512], f16, tag="t2", name="t2")
                    nc.vector.tensor_mul(t2[:], pq[:], sin_ap)
                    nc.vector.tensor_add(dst, t1[:], t2[:])

                def rope_chain(dst, w_name, bias_col, rhs, cos_ap, sin_ap,
                               dst_sl):
                    ps = pp.tile([P, 512], f32, tag="ps", name="ps")
                    for k in range(NT):
                        nc.tensor.matmul(ps[:], w_sb[w_name][:, k, dst_sl],
                                         rhs(k), start=(k == 0),
                                         stop=(k == NT - 1))
                    # psum->sbuf f16 with fused per-partition bias (ScalarE)
                    raw = sc.tile([P, 512], f16, tag="raw", name="raw")
                    nc.scalar.activation(raw[:], ps[:], AF.Identity,
                                         bias=bias_col)
                    pending.append((dst, raw, cos_ap, sin_ap))
                    if len(pending) > 1:
                        flush_one()

                def emit_q(t):
                    dst_sl = slice(P * t, P * (t + 1))
                    rope_chain(qt[:, t, :], "wq", bqt[:, t:t + 1],
                               lambda k: xqT[:, k, :], cosq[:], sinq[:],
                               dst_sl)

                def emit_k(t, n):
                    dst_sl = slice(P * t, P * (t + 1))
                    csl = slice(512 * n, 512 * (n + 1))
                    rope_chain(kt[:, t, csl], "wk", bkt[:, t:t + 1],
                               lambda k, csl=csl: xT[:, k, csl],
                               cosk[:, csl], sink[:, csl], dst_sl)

                def emit_v(t):
                    # V tile t (s-tile): natural [s, dout] into 65-wide slots
                    dst_sl = slice(P * t, P * (t + 1))
                    for n in range(2):
                        csl = slice(512 * n, 512 * (n + 1))
                        vp = pp.tile([P, 512], f32, tag="ps", name="vp")
                        for k in range(NT):
                            nc.tensor.matmul(vp[:], xT[:, k, dst_sl],
                                             w_sb["wv"][:, k, csl],
                                             start=(k == 0),
                                             stop=(k == NT - 1))
                        nc.vector.tensor_add(
                            v1r[:, t, 8 * n:8 * n + 8, 0:64],
                            vp.rearrange("p (h c) -> p h c", c=64),
                            bvb.rearrange("p (n h c) -> p n h c",
                                          n=2, c=64)[:, n])

                def emit_attn(p):
                    # attention head pair p (heads 2p, 2p+1); ctx+denom
                    # fused: stationary is the 65-wide [V|1] slot, PSUM
                    # rows 0:64 accumulate ctx and row 64 the denominator
                    cxA = cxp.tile([VS, 512], f32, tag="cxA", name="cxA")
                    cxB = cxp.tile([VS, 512], f32, tag="cxB", name="cxB")
                    h0, h1 = 2 * p, 2 * p + 1
                    es = {}

                    def emit_scores(j):
                        N = NJ[j]
                        co = 512 - N
                        e = ep.tile([P, 1024], f16, tag="e",
                                    name=f"e{p}_{j}")
                        for h in range(2):
                            rsl = slice(64 * h, 64 * (h + 1))
                            s_ps = scp.tile([P, 512], f32, tag="s",
                                            name=f"s{p}_{j}_{h}")
                            nc.tensor.matmul(s_ps[:, 0:N],
                                             kt[rsl, p, P * j:P * (j + 1)],
                                             qt[rsl, p, co:512],
                                             start=True, stop=True,
                                             skip_group_check=True)
                            nc.scalar.activation(e[:, 512 * h:512 * h + N],
                                                 s_ps[:, 0:N],
                                                 AF.Exp, scale=0.125)
                            nc.vector.tensor_mul(
                                e[:, 512 * h:512 * h + 64],
                                e[:, 512 * h:512 * h + 64], mj[:])
                        es[j] = e

                    def emit_ctx(j):
                        N = NJ[j]
                        co = 512 - N
                        e = es.pop(j)
                        st, sp = (j == 0), (j == NT - 1)
                        nc.tensor.matmul(cxA[:, co:512],
                                         v1[:, j, VS * h0:VS * h0 + VS],
                                         e[:, 0:N], start=st, stop=sp)
                        nc.tensor.matmul(cxB[:, co:512],
                                         v1[:, j, VS * h1:VS * h1 + VS],
                                         e[:, 512:512 + N],
                                         start=st, stop=sp)

                    # depth-2 software pipeline: scores run ahead of ctx
                    for j in range(NT + 2):
                        if j < NT:
                            emit_scores(j)
                        if j >= 2:
                            emit_ctx(j - 2)

                    # normalize: denom rows -> one [1,1024] copy pair, one
                    # recip, gpsimd broadcast, per-head multiply
                    dd = npl.tile([1, 1024], f32, tag="d", name="dd")
                    nc.vector.tensor_copy(dd[:, 0:512], cxA[64:65, :])
                    nc.vector.tensor_copy(dd[:, 512:1024], cxB[64:65, :])
                    rr = npl.tile([1, 1024], f32, tag="r", name="rr")
                    nc.vector.reciprocal_approx_fast(rr[:], dd[:])
                    rbs0 = npl.tile([64, 512], f32, tag="rb", name="rbs0")
                    nc.gpsimd.partition_broadcast(rbs0[:], rr[:, 0:512],
                                                  channels=64)
                    rbs1 = npl.tile([64, 512], f32, tag="rb", name="rbs1")
                    nc.gpsimd.partition_broadcast(rbs1[:], rr[:, 512:1024],
                                                  channels=64)
                    nc.vector.tensor_mul(cn[0:64, p, :], cxA[0:64, :],
                                         rbs0[:])
                    nc.vector.tensor_mul(cn[64:P, p, :], cxB[0:64, :],
                                         rbs1[:])

                # Dummy matmuls at the head of the PE queue: keep the PE
                # array busy while the first weight DMAs land so the HAM
                # clock gate opens (K=8) before the real chains start.
                warm = sc.tile([P, 512], f16, tag="warm", name="warm")
                nc.vector.memset(warm[:], 0.0)
                for i in range(40):
                    wp = pp.tile([P, 512], f32, tag="ps", name="wp")
                    nc.tensor.matmul(wp[:], warm[:, 0:P], warm[:],
                                     start=True, stop=True)

                # Emission order tracks DMA arrival: Q chains first (xqT +
                # wq first half), then K/V staged by xT halves, then the
                # t=2..7 groups with attention pair t-2 interleaved after
                # each (pairs 6 and 7 are held back for out-proj overlap).
                for t in range(4):
                    emit_q(t)
                emit_k(0, 0)
                emit_k(1, 0)
                for t in range(4):
                    emit_v(t)
                emit_k(0, 1)
                emit_k(1, 1)
                for t in range(4, NT):
                    emit_v(t)
                for t in range(2, NT):
                    if t >= 4:
                        emit_q(t)
                    emit_k(t, 0)
                    emit_k(t, 1)
                    emit_attn(t - 2)
                while pending:
                    flush_one()
                # tail: 3 out-proj chains start under the last attention
                # pairs (pp banks are free once the rope flushes drain)
                ypart = []

                def yp_chain_start(i, n, tmax):
                    csl = slice(512 * n, 512 * (n + 1))
                    yp = pp.tile([P, 512], f32, tag="ps",
                                 name=f"yp{i}_{n}")
                    for t in range(tmax):
                        nc.tensor.matmul(yp[:], cn[:, t, P * i:P * (i + 1)],
                                         w_sb["wo"][:, t, csl],
                                         start=(t == 0), stop=False)
                    ypart.append((yp, i, n))

                def yp_chain_extend(idx, t0, t1, stop):
                    yp, i, n = ypart[idx]
                    csl = slice(512 * n, 512 * (n + 1))
                    for t in range(t0, t1):
                        nc.tensor.matmul(yp[:], cn[:, t, P * i:P * (i + 1)],
                                         w_sb["wo"][:, t, csl],
                                         start=False,
                                         stop=(stop and t == t1 - 1))

                def yp_evac(yp, i, n):
                    csl = slice(512 * n, 512 * (n + 1))
                    ys = sc.tile([P, 512], f16, tag="ys", name="ys")
                    nc.vector.tensor_add(ys[:], yp[:], bob[:, csl])
                    nc.sync.dma_start(y_sh[P * i:P * (i + 1), csl], ys[:])

                for i, n in ((0, 0), (0, 1), (1, 0)):
                    yp_chain_start(i, n, 6)
                emit_attn(NT - 2)
                for idx in range(3):
                    yp_chain_extend(idx, 6, 7, stop=False)
                emit_attn(NT - 1)
                for idx in range(3):
                    yp_chain_extend(idx, 7, 8, stop=True)
                    yp_evac(*ypart[idx])
                # remaining chains alternate pp/scp banks so the PE never
                # waits on a single evacuation
                for idx, (i, n) in enumerate(((1, 1), (2, 0), (2, 1),
                                              (3, 0), (3, 1))):
                    pool, tag = (pp, "ps") if idx % 2 == 0 else (scp, "s")
                    yp = pool.tile([P, 512], f32, tag=tag,
                                   name=f"yp{i}_{n}")
                    csl = slice(512 * n, 512 * (n + 1))
                    for t in range(NT):
                        nc.tensor.matmul(yp[:], cn[:, t, P * i:P * (i + 1)],
                                         w_sb["wo"][:, t, csl],
                                         start=(t == 0),
                                         stop=(t == NT - 1))
                    yp_evac(yp, i, n)

            if taps:
                for tn, tile_ap in (("qt", qt), ("kt", kt), ("v1", v1),
                                    ("cn", cn)):
                    nc.sync.dma_start(tap_ext[tn][:], tile_ap[:])

    nc.compile()
    return nc


def _host_tables():
    # RoPE tables, computed in float32 to match the reference's jnp path.
    pos = np.arange(S, dtype=np.float32)
    inv = np.exp(np.arange(0, Dh, 2, dtype=np.float32)
                 * np.float32(-np.log(10000.0) / Dh))          # [32]
    ang = pos[:, None] * inv[None, :]                          # [S, 32]
    sin = np.sin(ang).astype(np.float32)
    cos = np.cos(ang).astype(np.float32)
    # per-partition pattern for [2 heads x 64, s] transposed layout
    dd = np.arange(P) % Dh
    cosP = np.empty((P, S), np.float32)
    sinP = np.empty((P, S), np.float32)
    lo = dd < 32
    cosP[lo] = cos[:, dd[lo]].T
    sinP[lo] = -sin[:, dd[lo]].T
    cosP[~lo] = cos[:, dd[~lo] - 32].T
    sinP[~lo] = sin[:, dd[~lo] - 32].T
    return cosP.astype(np.float16), sinP.astype(np.float16)


def _perm128():
    p = np.zeros((P, P), np.float16)
    i = np.arange(P)
    p[i, i ^ 32] = np.float16(1.0)
    return p


def _tile_T(a):
    # [rows, D] -> [P, NT, rows]: partition-tiled transpose for SBUF layout
    rows = a.shape[0]
    return np.ascontiguousarray(a.T.reshape(NT, P, rows).transpose(1, 0, 2))


def make_in_maps(x, Wq, bq, Wk, bk, Wv, bv, Wo, bo):
    x = np.asarray(x, np.float16)
    shared = {
        "wq": np.ascontiguousarray(np.asarray(Wq, np.float16)),
        "wk": np.ascontiguousarray(np.asarray(Wk, np.float16)),
        "wv": np.ascontiguousarray(np.asarray(Wv, np.float16)),
        "wo": np.ascontiguousarray(np.asarray(Wo, np.float16)),
        "bqt": np.ascontiguousarray(
            np.asarray(bq, np.float16).astype(np.float32).reshape(NT, P).T),
        "bkt": np.ascontiguousarray(
            np.asarray(bk, np.float16).astype(np.float32).reshape(NT, P).T),
        "bvb": np.ascontiguousarray(np.broadcast_to(
            np.asarray(bv, np.float16).reshape(1, D), (P, D))),
        "bob": np.ascontiguousarray(np.broadcast_to(
            np.asarray(bo, np.float16).reshape(1, D), (P, D))),
        "p128": _perm128(),
    }
    cosP, sinP = _host_tables()
    shared["cosk"] = cosP
    shared["sink"] = sinP

    in_maps = []
    r = np.arange(P)[:, None]
    cc2 = 2 * np.arange(64)[None, :]
    for core in range(N_CORES):
        b, par = core // 2, core % 2
        xb = x[b]                                   # [1024, 1024]
        # plain q order: col c <-> seq 2c+par
        xq = xb[par::2]                             # [512, 1024]
        m = {
            "xt_sh": _tile_T(xb),
            "xqt_sh": _tile_T(xq),
            "cosq": np.ascontiguousarray(cosP[:, par::2]),
            "sinq": np.ascontiguousarray(sinP[:, par::2]),
            "mj": (r <= cc2 + par).astype(np.float16),
        }
        m.update(shared)
        in_maps.append(m)
    return in_maps


def kernel(x, Wq, bq, Wk, bk, Wv, bv, Wo, bo):
    from concourse.bass_utils import run_bass_kernel_spmd

    with _lock:
        if "nc" not in _cache:
            _cache["nc"] = _build_program()
    nc = _cache["nc"]

    in_maps = make_in_maps(x, Wq, bq, Wk, bk, Wv, bv, Wo, bo)
    res = run_bass_kernel_spmd(nc, in_maps, list(range(N_CORES)))

    out = np.empty((B, S, D), np.float16)
    o2 = out.reshape(B, 512, 2, D)
    for core in range(N_CORES):
        b, par = core // 2, core % 2
        o2[b, :, par, :] = res.results[core]["y_sh"]
    return out



# revision 33
# speedup vs baseline: 1.1423x; 1.1423x over previous
"""Trainium2 Bass kernel for CustomMultiHeadAttention (B=4, S=1024, D=1024, H=16, Dh=64).

Sharding: 8 cores = (batch b in 0..3) x (parity par in 0..1).
Core (b, par) computes output rows {s : s % 2 == par} of batch b,
grouped into 4 "vblocks" of 128 rows (vblock i' = seq 256*i' + 2*c + par).
K/V are computed for the full sequence on every core (from the full x[b]).
The program is identical on all cores; per-core differences are input data.

Pipeline (all transposed-layout, PE-centric):
  QT = rope(Wq^T x^T), KT = rope(Wk^T x^T)  - rope via permutation-matmul + DVE
  scT[kv,q] = KT_h^T QT_h (2 heads row-packed), exp on ScalarE (scale=1/8),
  causal mask = f16 0/1 multiply on the diagonal 128 cols,
  ctxT/denoms accumulate via lhsT=[V|1], normalize via reciprocal_approx_fast
  + PE broadcast, out = ctxT^T Wo.
"""

import threading

import numpy as np

B, S, D, H, Dh = 4, 1024, 1024, 16, 64
P = 128
N_CORES = 8
NT = D // P  # 8 tiles along d/dout/seq
# q col c <-> seq 2c+par (plain order); for kv-block j the active q
# cols are the suffix [64j, 512) and the mask covers its first 64 cols
NJ = [512 - 64 * j for j in range(8)]
VS = 65  # V slot width: [V(64) | ones(1)] per head

_cache = {}
_lock = threading.Lock()


def _build_program(taps=False):
    import concourse.bass as bass  # noqa: F401
    import concourse.mybir as mybir
    import concourse.tile as tile
    from concourse import bacc

    dt = mybir.dt
    f16, f32 = dt.float16, dt.float32
    AF = mybir.ActivationFunctionType

    nc = bacc.Bacc("TRN2", target_bir_lowering=False, debug=False,
                   num_devices=N_CORES)

    def ein(name, shape):
        return nc.dram_tensor(name, shape, f16, kind="ExternalInput").ap()

    xt_sh = ein("xt_sh", [P, NT, S])     # x[b]^T, host-transposed
    xqt_sh = ein("xqt_sh", [P, NT, 512])  # xq^T, host-transposed
    w_ext = {n: ein(n, [D, D]) for n in ("wq", "wk", "wv", "wo")}
    bqt_e = nc.dram_tensor("bqt", [P, NT], f32, kind="ExternalInput").ap()
    bkt_e = nc.dram_tensor("bkt", [P, NT], f32, kind="ExternalInput").ap()
    bvb_e = ein("bvb", [P, D])           # bv broadcast across partitions
    bob_e = ein("bob", [P, D])           # bo broadcast across partitions
    cosq_e = ein("cosq", [P, 512])
    sinq_e = ein("sinq", [P, 512])
    cosk_e = ein("cosk", [P, S])
    sink_e = ein("sink", [P, S])
    mj_e = ein("mj", [P, 64])
    p128_e = ein("p128", [P, P])
    y_sh = nc.dram_tensor("y_sh", [512, D], f16, kind="ExternalOutput").ap()
    tap_ext = {}
    if taps:
        for tn, shape in (("qt", [P, NT, 512]), ("kt", [P, NT, S]),
                          ("v1", [P, NT, H * VS]), ("cn", [P, NT, 512])):
            tap_ext[tn] = nc.dram_tensor("dbg_" + tn, shape, f16,
                                         kind="ExternalOutput").ap()

    with tile.TileContext(nc) as tc:
        from contextlib import ExitStack
        with ExitStack() as ctx:
            big = ctx.enter_context(tc.tile_pool(name="big", bufs=1))

            xT = big.tile([P, NT, S], f16, tag="xT")        # x[b]^T  [din, s]
            xqT = big.tile([P, NT, 512], f16, tag="xqT")    # xq^T    [din, q]
            w_sb = {n: big.tile([P, NT, D], f16, tag=n, name=n + "_sb")
                    for n in w_ext}
            bqt = big.tile([P, NT], f32, tag="bqt")
            bkt = big.tile([P, NT], f32, tag="bkt")
            bvb = big.tile([P, D], f16, tag="bvb")
            bob = big.tile([P, D], f16, tag="bob")
            qt = big.tile([P, NT, 512], f16, tag="qt")      # rope'd Q^T
            kt = big.tile([P, NT, S], f16, tag="kt")        # rope'd K^T
            v1 = big.tile([P, NT, H * VS], f16, tag="v1")   # [V|1] slots
            cn = big.tile([P, NT, 512], f16, tag="cn")      # normalized ctx^T
            cosq = big.tile([P, 512], f16, tag="cosq")
            sinq = big.tile([P, 512], f16, tag="sinq")
            cosk = big.tile([P, S], f16, tag="cosk")
            sink = big.tile([P, S], f16, tag="sink")
            mj = big.tile([P, 64], f16, tag="mj")
            p128 = big.tile([P, P], f16, tag="p128")

            # ---- input DMAs ----
            # Coarse DMAs: one dma_start per tensor chunk (issue overhead
            # is ~0.6us each, so fewer+bigger wins). Order on the sync
            # queue matches first-use order of the projection chains.
            def wslice(name, c0, c1):
                src = w_ext[name].rearrange("(k p) d -> p k d", p=P)
                nc.sync.dma_start(w_sb[name][:, :, c0:c1],
                                  src[:, :, c0:c1])

            nc.sync.dma_start(xqT[:], xqt_sh[:])
            wslice("wq", 0, 512)
            for t, e in ((p128, p128_e), (cosq, cosq_e),
                         (sinq, sinq_e), (bqt, bqt_e)):
                nc.sync.dma_start(t[:], e[:])
            nc.sync.dma_start(xT[:, :, 0:512], xt_sh[:, :, 0:512])
            wslice("wk", 0, 512)
            for t, e in ((cosk, cosk_e), (sink, sink_e), (bkt, bkt_e)):
                nc.sync.dma_start(t[:], e[:])
            nc.sync.dma_start(xT[:, :, 512:1024], xt_sh[:, :, 512:1024])
            wslice("wq", 512, 1024)
            wslice("wk", 512, 1024)
            # bulk weights on the gpsimd queue, in parallel
            nc.gpsimd.dma_start(
                w_sb["wv"][:],
                w_ext["wv"].rearrange("(k p) d -> p k d", p=P))
            for t, e in ((bvb, bvb_e), (mj, mj_e)):
                nc.gpsimd.dma_start(t[:], e[:])
            nc.gpsimd.dma_start(
                w_sb["wo"][:],
                w_ext["wo"].rearrange("(k p) d -> p k d", p=P))
            nc.gpsimd.dma_start(bob[:], bob_e[:])
            # ones columns of the V slots (col 64 of each 65-wide slot)
            v1r = v1.rearrange("p t (h c) -> p t h c", c=VS)
            for t in range(NT):
                nc.any.memset(v1r[:, t, :, 64:65], 1.0)

            # ---- projections + rope + attention, interleaved ----
            # Projections use a single rotating PSUM tag (3 banks); the
            # perm-matmul for rope unit u-1 is emitted after unit u's chain
            # so the PE never waits on the ScalarE psum->sbuf evacuation.
            # Attention head-pair p is emitted right after the t=p+1
            # projections, so its ScalarE exp work overlaps the remaining
            # projection matmuls instead of serializing behind them.
            # PSUM banks: pp 3 + scp 3 + cxp 2 = 8.
            with tc.tile_pool(name="pp", bufs=3, space="PSUM") as pp, \
                 tc.tile_pool(name="sc", bufs=4) as sc, \
                 tc.tile_pool(name="scp", bufs=3, space="PSUM") as scp, \
                 tc.tile_pool(name="cxp", bufs=1, space="PSUM") as cxp, \
                 tc.tile_pool(name="ep", bufs=3) as ep, \
                 tc.tile_pool(name="npl", bufs=2) as npl:
                pending = []

                def flush_one():
                    dst, raw, cos_ap, sin_ap = pending.pop(0)
                    pq = pp.tile([P, 512], f32, tag="ps", name="pq")
                    nc.tensor.matmul(pq[:], p128[:], raw[:],
                                     start=True, stop=True)
                    t1 = sc.tile([P, 512], f16, tag="t1", name="t1")
                    nc.vector.tensor_mul(t1[:], raw[:], cos_ap)
                    t2 = sc.tile([P, 512], f16, tag="t2", name="t2")
                    nc.vector.tensor_mul(t2[:], pq[:], sin_ap)
                    nc.vector.tensor_add(dst, t1[:], t2[:])

                def rope_chain(dst, w_name, bias_col, rhs, cos_ap, sin_ap,
                               dst_sl):
                    ps = pp.tile([P, 512], f32, tag="ps", name="ps")
                    for k in range(NT):
                        nc.tensor.matmul(ps[:], w_sb[w_name][:, k, dst_sl],
                                         rhs(k), start=(k == 0),
                                         stop=(k == NT - 1))
                    # psum->sbuf f16 with fused per-partition bias (ScalarE)
                    raw = sc.tile([P, 512], f16, tag="raw", name="raw")
                    nc.scalar.activation(raw[:], ps[:], AF.Identity,
                                         bias=bias_col)
                    pending.append((dst, raw, cos_ap, sin_ap))
                    if len(pending) > 1:
                        flush_one()

                def emit_q(t):
                    dst_sl = slice(P * t, P * (t + 1))
                    rope_chain(qt[:, t, :], "wq", bqt[:, t:t + 1],
                               lambda k: xqT[:, k, :], cosq[:], sinq[:],
                               dst_sl)

                def emit_k(t, n):
                    dst_sl = slice(P * t, P * (t + 1))
                    csl = slice(512 * n, 512 * (n + 1))
                    rope_chain(kt[:, t, csl], "wk", bkt[:, t:t + 1],
                               lambda k, csl=csl: xT[:, k, csl],
                               cosk[:, csl], sink[:, csl], dst_sl)

                def emit_v(t):
                    # V tile t (s-tile): natural [s, dout] into 65-wide slots
                    dst_sl = slice(P * t, P * (t + 1))
                    for n in range(2):
                        csl = slice(512 * n, 512 * (n + 1))
                        vp = pp.tile([P, 512], f32, tag="ps", name="vp")
                        for k in range(NT):
                            nc.tensor.matmul(vp[:], xT[:, k, dst_sl],
                                             w_sb["wv"][:, k, csl],
                                             start=(k == 0),
                                             stop=(k == NT - 1))
                        nc.vector.tensor_add(
                            v1r[:, t, 8 * n:8 * n + 8, 0:64],
                            vp.rearrange("p (h c) -> p h c", c=64),
                            bvb.rearrange("p (n h c) -> p n h c",
                                          n=2, c=64)[:, n])

                def emit_attn(p):
                    # attention head pair p (heads 2p, 2p+1); ctx+denom
                    # fused: stationary is the 65-wide [V|1] slot, PSUM
                    # rows 0:64 accumulate ctx and row 64 the denominator
                    cxA = cxp.tile([VS, 512], f32, tag="cxA", name="cxA")
                    cxB = cxp.tile([VS, 512], f32, tag="cxB", name="cxB")
                    h0, h1 = 2 * p, 2 * p + 1
                    es = {}

                    def emit_scores(j):
                        N = NJ[j]
                        co = 512 - N
                        e = ep.tile([P, 1024], f16, tag="e",
                                    name=f"e{p}_{j}")
                        for h in range(2):
                            rsl = slice(64 * h, 64 * (h + 1))
                            s_ps = scp.tile([P, 512], f32, tag="s",
                                            name=f"s{p}_{j}_{h}")
                            nc.tensor.matmul(s_ps[:, 0:N],
                                             kt[rsl, p, P * j:P * (j + 1)],
                                             qt[rsl, p, co:512],
                                             start=True, stop=True,
                                             skip_group_check=True)
                            nc.scalar.activation(e[:, 512 * h:512 * h + N],
                                                 s_ps[:, 0:N],
                                                 AF.Exp, scale=0.125)
                            nc.vector.tensor_mul(
                                e[:, 512 * h:512 * h + 64],
                                e[:, 512 * h:512 * h + 64], mj[:])
                        es[j] = e

                    def emit_ctx(j):
                        N = NJ[j]
                        co = 512 - N
                        e = es.pop(j)
                        st, sp = (j == 0), (j == NT - 1)
                        nc.tensor.matmul(cxA[:, co:512],
                                         v1[:, j, VS * h0:VS * h0 + VS],
                                         e[:, 0:N], start=st, stop=sp)
                        nc.tensor.matmul(cxB[:, co:512],
                                         v1[:, j, VS * h1:VS * h1 + VS],
                                         e[:, 512:512 + N],
                                         start=st, stop=sp)

                    # depth-2 software pipeline: scores run ahead of ctx
                    for j in range(NT + 2):
                        if j < NT:
                            emit_scores(j)
                        if j >= 2:
                            emit_ctx(j - 2)

                    # normalize: denom rows -> one [1,1024] copy pair, one
                    # recip, gpsimd broadcast, per-head multiply
                    dd = npl.tile([1, 1024], f32, tag="d", name="dd")
                    nc.vector.tensor_copy(dd[:, 0:512], cxA[64:65, :])
                    nc.vector.tensor_copy(dd[:, 512:1024], cxB[64:65, :])
                    rr = npl.tile([1, 1024], f32, tag="r", name="rr")
                    nc.vector.reciprocal_approx_fast(rr[:], dd[:])
                    rbs0 = npl.tile([64, 512], f32, tag="rb", name="rbs0")
                    nc.gpsimd.partition_broadcast(rbs0[:], rr[:, 0:512],
                                                  channels=64)
                    rbs1 = npl.tile([64, 512], f32, tag="rb", name="rbs1")
                    nc.gpsimd.partition_broadcast(rbs1[:], rr[:, 512:1024],
                                                  channels=64)
                    nc.vector.tensor_mul(cn[0:64, p, :], cxA[0:64, :],
                                         rbs0[:])
                    nc.vector.tensor_mul(cn[64:P, p, :], cxB[0:64, :],
                                         rbs1[:])

                # Dummy matmuls at the head of the PE queue: keep the PE
                # array busy while the first weight DMAs land so the HAM
                # clock gate opens (K=8) before the real chains start.
                warm = sc.tile([P, 512], f16, tag="warm", name="warm")
                nc.vector.memset(warm[:], 0.0)
                for i in range(40):
                    wp = pp.tile([P, 512], f32, tag="ps", name="wp")
                    nc.tensor.matmul(wp[:], warm[:, 0:P], warm[:],
                                     start=True, stop=True)

                # Emission order tracks DMA arrival: Q chains first (xqT +
                # wq first half), then K/V staged by xT halves, then the
                # t=2..7 groups with attention pair t-2 interleaved after
                # each (pairs 6 and 7 are held back for out-proj overlap).
                for t in range(4):
                    emit_q(t)
                emit_k(0, 0)
                emit_k(1, 0)
                for t in range(4):
                    emit_v(t)
                emit_k(0, 1)
                emit_k(1, 1)
                for t in range(4, NT):
                    emit_v(t)
                emit_attn(0)
                for t in range(2, NT):
                    if t >= 4:
                        emit_q(t)
                    emit_k(t, 0)
                    emit_k(t, 1)
                    emit_attn(t - 1)
                while pending:
                    flush_one()
                # tail: 3 out-proj chains start under the last attention
                # pairs (pp banks are free once the rope flushes drain)
                ypart = []

                def yp_chain_start(i, n, tmax):
                    csl = slice(512 * n, 512 * (n + 1))
                    yp = pp.tile([P, 512], f32, tag="ps",
                                 name=f"yp{i}_{n}")
                    for t in range(tmax):
                        nc.tensor.matmul(yp[:], cn[:, t, P * i:P * (i + 1)],
                                         w_sb["wo"][:, t, csl],
                                         start=(t == 0), stop=False)
                    ypart.append((yp, i, n))

                def yp_chain_extend(idx, t0, t1, stop):
                    yp, i, n = ypart[idx]
                    csl = slice(512 * n, 512 * (n + 1))
                    for t in range(t0, t1):
                        nc.tensor.matmul(yp[:], cn[:, t, P * i:P * (i + 1)],
                                         w_sb["wo"][:, t, csl],
                                         start=False,
                                         stop=(stop and t == t1 - 1))

                def yp_evac(yp, i, n):
                    csl = slice(512 * n, 512 * (n + 1))
                    ys = sc.tile([P, 512], f16, tag="ys", name="ys")
                    nc.vector.tensor_add(ys[:], yp[:], bob[:, csl])
                    nc.sync.dma_start(y_sh[P * i:P * (i + 1), csl], ys[:])

                for i, n in ((0, 0), (0, 1), (1, 0)):
                    yp_chain_start(i, n, 6)
                emit_attn(NT - 2)
                for idx in range(3):
                    yp_chain_extend(idx, 6, 7, stop=False)
                emit_attn(NT - 1)
                for idx in range(3):
                    yp_chain_extend(idx, 7, 8, stop=True)
                    yp_evac(*ypart[idx])
                # remaining chains alternate pp/scp banks so the PE never
                # waits on a single evacuation
                for idx, (i, n) in enumerate(((1, 1), (2, 0), (2, 1),
                                              (3, 0), (3, 1))):
                    pool, tag = (pp, "ps") if idx % 2 == 0 else (scp, "s")
                    yp = pool.tile([P, 512], f32, tag=tag,
                                   name=f"yp{i}_{n}")
                    csl = slice(512 * n, 512 * (n + 1))
                    for t in range(NT):
                        nc.tensor.matmul(yp[:], cn[:, t, P * i:P * (i + 1)],
                                         w_sb["wo"][:, t, csl],
                                         start=(t == 0),
                                         stop=(t == NT - 1))
                    yp_evac(yp, i, n)

            if taps:
                for tn, tile_ap in (("qt", qt), ("kt", kt), ("v1", v1),
                                    ("cn", cn)):
                    nc.sync.dma_start(tap_ext[tn][:], tile_ap[:])

    nc.compile()
    return nc


def _host_tables():
    # RoPE tables, computed in float32 to match the reference's jnp path.
    pos = np.arange(S, dtype=np.float32)
    inv = np.exp(np.arange(0, Dh, 2, dtype=np.float32)
                 * np.float32(-np.log(10000.0) / Dh))          # [32]
    ang = pos[:, None] * inv[None, :]                          # [S, 32]
    sin = np.sin(ang).astype(np.float32)
    cos = np.cos(ang).astype(np.float32)
    # per-partition pattern for [2 heads x 64, s] transposed layout
    dd = np.arange(P) % Dh
    cosP = np.empty((P, S), np.float32)
    sinP = np.empty((P, S), np.float32)
    lo = dd < 32
    cosP[lo] = cos[:, dd[lo]].T
    sinP[lo] = -sin[:, dd[lo]].T
    cosP[~lo] = cos[:, dd[~lo] - 32].T
    sinP[~lo] = sin[:, dd[~lo] - 32].T
    return cosP.astype(np.float16), sinP.astype(np.float16)


def _perm128():
    p = np.zeros((P, P), np.float16)
    i = np.arange(P)
    p[i, i ^ 32] = np.float16(1.0)
    return p


def _tile_T(a):
    # [rows, D] -> [P, NT, rows]: partition-tiled transpose for SBUF layout
    rows = a.shape[0]
    return np.ascontiguousarray(a.T.reshape(NT, P, rows).transpose(1, 0, 2))


def make_in_maps(x, Wq, bq, Wk, bk, Wv, bv, Wo, bo):
    x = np.asarray(x, np.float16)
    shared = {
        "wq": np.ascontiguousarray(np.asarray(Wq, np.float16)),
        "wk": np.ascontiguousarray(np.asarray(Wk, np.float16)),
        "wv": np.ascontiguousarray(np.asarray(Wv, np.float16)),
        "wo": np.ascontiguousarray(np.asarray(Wo, np.float16)),
        "bqt": np.ascontiguousarray(
            np.asarray(bq, np.float16).astype(np.float32).reshape(NT, P).T),
        "bkt": np.ascontiguousarray(
            np.asarray(bk, np.float16).astype(np.float32).reshape(NT, P).T),
        "bvb": np.ascontiguousarray(np.broadcast_to(
            np.asarray(bv, np.float16).reshape(1, D), (P, D))),
        "bob": np.ascontiguousarray(np.broadcast_to(
            np.asarray(bo, np.float16).reshape(1, D), (P, D))),
        "p128": _perm128(),
    }
    cosP, sinP = _host_tables()
    shared["cosk"] = cosP
    shared["sink"] = sinP

    in_maps = []
    r = np.arange(P)[:, None]
    cc2 = 2 * np.arange(64)[None, :]
    for core in range(N_CORES):
        b, par = core // 2, core % 2
        xb = x[b]                                   # [1024, 1024]
        # plain q order: col c <-> seq 2c+par
        xq = xb[par::2]                             # [512, 1024]
        m = {
            "xt_sh": _tile_T(xb),
            "xqt_sh": _tile_T(xq),
            "cosq": np.ascontiguousarray(cosP[:, par::2]),
            "sinq": np.ascontiguousarray(sinP[:, par::2]),
            "mj": (r <= cc2 + par).astype(np.float16),
        }
        m.update(shared)
        in_maps.append(m)
    return in_maps


def kernel(x, Wq, bq, Wk, bk, Wv, bv, Wo, bo):
    from concourse.bass_utils import run_bass_kernel_spmd

    with _lock:
        if "nc" not in _cache:
            _cache["nc"] = _build_program()
    nc = _cache["nc"]

    in_maps = make_in_maps(x, Wq, bq, Wk, bk, Wv, bv, Wo, bo)
    res = run_bass_kernel_spmd(nc, in_maps, list(range(N_CORES)))

    out = np.empty((B, S, D), np.float16)
    o2 = out.reshape(B, 512, 2, D)
    for core in range(N_CORES):
        b, par = core // 2, core % 2
        o2[b, :, par, :] = res.results[core]["y_sh"]
    return out



# revision 34
# speedup vs baseline: 1.2377x; 1.0835x over previous
"""Trainium2 Bass kernel for CustomMultiHeadAttention (B=4, S=1024, D=1024, H=16, Dh=64).

Sharding: 8 cores = (batch b in 0..3) x (parity par in 0..1).
Core (b, par) computes output rows {s : s % 2 == par} of batch b,
grouped into 4 "vblocks" of 128 rows (vblock i' = seq 256*i' + 2*c + par).
K/V are computed for the full sequence on every core (from the full x[b]).
The program is identical on all cores; per-core differences are input data.

Pipeline (all transposed-layout, PE-centric):
  QT = rope(Wq^T x^T), KT = rope(Wk^T x^T)  - rope via permutation-matmul + DVE
  scT[kv,q] = KT_h^T QT_h (2 heads row-packed), exp on ScalarE (scale=1/8),
  causal mask = f16 0/1 multiply on the diagonal 128 cols,
  ctxT/denoms accumulate via lhsT=[V|1], normalize via reciprocal_approx_fast
  + PE broadcast, out = ctxT^T Wo.
"""

import threading

import numpy as np

B, S, D, H, Dh = 4, 1024, 1024, 16, 64
P = 128
N_CORES = 8
NT = D // P  # 8 tiles along d/dout/seq
# q col c <-> seq 2c+par (plain order); for kv-block j the active q
# cols are the suffix [64j, 512) and the mask covers its first 64 cols
NJ = [512 - 64 * j for j in range(8)]
VS = 65  # V slot width: [V(64) | ones(1)] per head

_cache = {}
_lock = threading.Lock()


def _build_program(taps=False):
    import concourse.bass as bass  # noqa: F401
    import concourse.mybir as mybir
    import concourse.tile as tile
    from concourse import bacc

    dt = mybir.dt
    f16, f32 = dt.float16, dt.float32
    AF = mybir.ActivationFunctionType

    nc = bacc.Bacc("TRN2", target_bir_lowering=False, debug=False,
                   num_devices=N_CORES)

    def ein(name, shape):
        return nc.dram_tensor(name, shape, f16, kind="ExternalInput").ap()

    xt_sh = ein("xt_sh", [P, NT, S])     # x[b]^T, host-transposed
    xqt_sh = ein("xqt_sh", [P, NT, 512])  # xq^T, host-transposed
    w_ext = {n: ein(n, [D, D]) for n in ("wq", "wk", "wv", "wo")}
    bqt_e = nc.dram_tensor("bqt", [P, NT], f32, kind="ExternalInput").ap()
    bkt_e = nc.dram_tensor("bkt", [P, NT], f32, kind="ExternalInput").ap()
    bvb_e = ein("bvb", [P, D])           # bv broadcast across partitions
    bob_e = ein("bob", [P, D])           # bo broadcast across partitions
    cosq_e = ein("cosq", [P, 512])
    sinq_e = ein("sinq", [P, 512])
    cosk_e = ein("cosk", [P, S])
    sink_e = ein("sink", [P, S])
    mj_e = ein("mj", [P, 64])
    p128_e = ein("p128", [P, P])
    y_sh = nc.dram_tensor("y_sh", [512, D], f16, kind="ExternalOutput").ap()
    tap_ext = {}
    if taps:
        for tn, shape in (("qt", [P, NT, 512]), ("kt", [P, NT, S]),
                          ("v1", [P, NT, H * VS]), ("cn", [P, NT, 512])):
            tap_ext[tn] = nc.dram_tensor("dbg_" + tn, shape, f16,
                                         kind="ExternalOutput").ap()

    with tile.TileContext(nc) as tc:
        from contextlib import ExitStack
        with ExitStack() as ctx:
            big = ctx.enter_context(tc.tile_pool(name="big", bufs=1))

            xT = big.tile([P, NT, S], f16, tag="xT")        # x[b]^T  [din, s]
            xqT = big.tile([P, NT, 512], f16, tag="xqT")    # xq^T    [din, q]
            w_sb = {n: big.tile([P, NT, D], f16, tag=n, name=n + "_sb")
                    for n in w_ext}
            bqt = big.tile([P, NT], f32, tag="bqt")
            bkt = big.tile([P, NT], f32, tag="bkt")
            bvb = big.tile([P, D], f16, tag="bvb")
            bob = big.tile([P, D], f16, tag="bob")
            qt = big.tile([P, NT, 512], f16, tag="qt")      # rope'd Q^T
            kt = big.tile([P, NT, S], f16, tag="kt")        # rope'd K^T
            v1 = big.tile([P, NT, H * VS], f16, tag="v1")   # [V|1] slots
            cn = big.tile([P, NT, 512], f16, tag="cn")      # normalized ctx^T
            cosq = big.tile([P, 512], f16, tag="cosq")
            sinq = big.tile([P, 512], f16, tag="sinq")
            cosk = big.tile([P, S], f16, tag="cosk")
            sink = big.tile([P, S], f16, tag="sink")
            mj = big.tile([P, 64], f16, tag="mj")
            p128 = big.tile([P, P], f16, tag="p128")

            # ---- input DMAs ----
            # Coarse DMAs: one dma_start per tensor chunk (issue overhead
            # is ~0.6us each, so fewer+bigger wins). Order on the sync
            # queue matches first-use order of the projection chains.
            def wslice(name, c0, c1):
                src = w_ext[name].rearrange("(k p) d -> p k d", p=P)
                nc.sync.dma_start(w_sb[name][:, :, c0:c1],
                                  src[:, :, c0:c1])

            nc.sync.dma_start(xqT[:], xqt_sh[:])
            wslice("wq", 0, 512)
            for t, e in ((p128, p128_e), (cosq, cosq_e),
                         (sinq, sinq_e), (bqt, bqt_e)):
                nc.sync.dma_start(t[:], e[:])
            nc.sync.dma_start(xT[:, :, 0:512], xt_sh[:, :, 0:512])
            wslice("wk", 0, 512)
            for t, e in ((cosk, cosk_e), (sink, sink_e), (bkt, bkt_e)):
                nc.sync.dma_start(t[:], e[:])
            nc.sync.dma_start(xT[:, :, 512:1024], xt_sh[:, :, 512:1024])
            wslice("wq", 512, 1024)
            wslice("wk", 512, 1024)
            # bulk weights on the gpsimd queue, in parallel
            nc.gpsimd.dma_start(
                w_sb["wv"][:],
                w_ext["wv"].rearrange("(k p) d -> p k d", p=P))
            for t, e in ((bvb, bvb_e), (mj, mj_e)):
                nc.gpsimd.dma_start(t[:], e[:])
            nc.gpsimd.dma_start(
                w_sb["wo"][:],
                w_ext["wo"].rearrange("(k p) d -> p k d", p=P))
            nc.gpsimd.dma_start(bob[:], bob_e[:])
            # ones columns of the V slots (col 64 of each 65-wide slot)
            v1r = v1.rearrange("p t (h c) -> p t h c", c=VS)
            for t in range(NT):
                nc.any.memset(v1r[:, t, :, 64:65], 1.0)

            # ---- projections + rope + attention, interleaved ----
            # Projections use a single rotating PSUM tag (3 banks); the
            # perm-matmul for rope unit u-1 is emitted after unit u's chain
            # so the PE never waits on the ScalarE psum->sbuf evacuation.
            # Attention head-pair p is emitted right after the t=p+1
            # projections, so its ScalarE exp work overlaps the remaining
            # projection matmuls instead of serializing behind them.
            # PSUM banks: pp 3 + scp 3 + cxp 2 = 8.
            with tc.tile_pool(name="pp", bufs=3, space="PSUM") as pp, \
                 tc.tile_pool(name="sc", bufs=4) as sc, \
                 tc.tile_pool(name="scp", bufs=3, space="PSUM") as scp, \
                 tc.tile_pool(name="cxp", bufs=1, space="PSUM") as cxp, \
                 tc.tile_pool(name="ep", bufs=3) as ep, \
                 tc.tile_pool(name="npl", bufs=2) as npl:
                pending = []

                def flush_one():
                    dst, raw, cos_ap, sin_ap = pending.pop(0)
                    pq = pp.tile([P, 512], f32, tag="ps", name="pq")
                    nc.tensor.matmul(pq[:], p128[:], raw[:],
                                     start=True, stop=True)
                    t1 = sc.tile([P, 512], f16, tag="t1", name="t1")
                    nc.vector.tensor_mul(t1[:], raw[:], cos_ap)
                    t2 = sc.tile([P, 512], f16, tag="t2", name="t2")
                    nc.vector.tensor_mul(t2[:], pq[:], sin_ap)
                    nc.vector.tensor_add(dst, t1[:], t2[:])

                def rope_chain(dst, w_name, bias_col, rhs, cos_ap, sin_ap,
                               dst_sl):
                    ps = pp.tile([P, 512], f32, tag="ps", name="ps")
                    for k in range(NT):
                        nc.tensor.matmul(ps[:], w_sb[w_name][:, k, dst_sl],
                                         rhs(k), start=(k == 0),
                                         stop=(k == NT - 1))
                    # psum->sbuf f16 with fused per-partition bias (ScalarE)
                    raw = sc.tile([P, 512], f16, tag="raw", name="raw")
                    nc.scalar.activation(raw[:], ps[:], AF.Identity,
                                         bias=bias_col)
                    pending.append((dst, raw, cos_ap, sin_ap))
                    if len(pending) > 1:
                        flush_one()

                def emit_q(t):
                    dst_sl = slice(P * t, P * (t + 1))
                    rope_chain(qt[:, t, :], "wq", bqt[:, t:t + 1],
                               lambda k: xqT[:, k, :], cosq[:], sinq[:],
                               dst_sl)

                def emit_k(t, n):
                    dst_sl = slice(P * t, P * (t + 1))
                    csl = slice(512 * n, 512 * (n + 1))
                    rope_chain(kt[:, t, csl], "wk", bkt[:, t:t + 1],
                               lambda k, csl=csl: xT[:, k, csl],
                               cosk[:, csl], sink[:, csl], dst_sl)

                def emit_v(t):
                    # V tile t (s-tile): natural [s, dout] into 65-wide slots
                    dst_sl = slice(P * t, P * (t + 1))
                    for n in range(2):
                        csl = slice(512 * n, 512 * (n + 1))
                        vp = pp.tile([P, 512], f32, tag="ps", name="vp")
                        for k in range(NT):
                            nc.tensor.matmul(vp[:], xT[:, k, dst_sl],
                                             w_sb["wv"][:, k, csl],
                                             start=(k == 0),
                                             stop=(k == NT - 1))
                        nc.vector.tensor_add(
                            v1r[:, t, 8 * n:8 * n + 8, 0:64],
                            vp.rearrange("p (h c) -> p h c", c=64),
                            bvb.rearrange("p (n h c) -> p n h c",
                                          n=2, c=64)[:, n])

                def emit_attn(p):
                    # attention head pair p (heads 2p, 2p+1); ctx+denom
                    # fused: stationary is the 65-wide [V|1] slot, PSUM
                    # rows 0:64 accumulate ctx and row 64 the denominator
                    cxA = cxp.tile([VS, 512], f32, tag="cxA", name="cxA")
                    cxB = cxp.tile([VS, 512], f32, tag="cxB", name="cxB")
                    h0, h1 = 2 * p, 2 * p + 1
                    es = {}

                    def emit_scores(j):
                        N = NJ[j]
                        co = 512 - N
                        e = ep.tile([P, 1024], f16, tag="e",
                                    name=f"e{p}_{j}")
                        for h in range(2):
                            rsl = slice(64 * h, 64 * (h + 1))
                            s_ps = scp.tile([P, 512], f32, tag="s",
                                            name=f"s{p}_{j}_{h}")
                            nc.tensor.matmul(s_ps[:, 0:N],
                                             kt[rsl, p, P * j:P * (j + 1)],
                                             qt[rsl, p, co:512],
                                             start=True, stop=True,
                                             skip_group_check=True)
                            nc.scalar.activation(e[:, 512 * h:512 * h + N],
                                                 s_ps[:, 0:N],
                                                 AF.Exp, scale=0.125)
                            nc.vector.tensor_mul(
                                e[:, 512 * h:512 * h + 64],
                                e[:, 512 * h:512 * h + 64], mj[:])
                        es[j] = e

                    def emit_ctx(j):
                        N = NJ[j]
                        co = 512 - N
                        e = es.pop(j)
                        st, sp = (j == 0), (j == NT - 1)
                        nc.tensor.matmul(cxA[:, co:512],
                                         v1[:, j, VS * h0:VS * h0 + VS],
                                         e[:, 0:N], start=st, stop=sp)
                        nc.tensor.matmul(cxB[:, co:512],
                                         v1[:, j, VS * h1:VS * h1 + VS],
                                         e[:, 512:512 + N],
                                         start=st, stop=sp)

                    # depth-2 software pipeline: scores run ahead of ctx
                    for j in range(NT + 2):
                        if j < NT:
                            emit_scores(j)
                        if j >= 2:
                            emit_ctx(j - 2)

                    # normalize: denom rows -> one [1,1024] copy pair, one
                    # recip, gpsimd broadcast, per-head multiply
                    dd = npl.tile([1, 1024], f32, tag="d", name="dd")
                    nc.vector.tensor_copy(dd[:, 0:512], cxA[64:65, :])
                    nc.vector.tensor_copy(dd[:, 512:1024], cxB[64:65, :])
                    rr = npl.tile([1, 1024], f32, tag="r", name="rr")
                    nc.vector.reciprocal_approx_fast(rr[:], dd[:])
                    rbs0 = npl.tile([64, 512], f32, tag="rb", name="rbs0")
                    nc.gpsimd.partition_broadcast(rbs0[:], rr[:, 0:512],
                                                  channels=64)
                    rbs1 = npl.tile([64, 512], f32, tag="rb", name="rbs1")
                    nc.gpsimd.partition_broadcast(rbs1[:], rr[:, 512:1024],
                                                  channels=64)
                    nc.vector.tensor_mul(cn[0:64, p, :], cxA[0:64, :],
                                         rbs0[:])
                    nc.vector.tensor_mul(cn[64:P, p, :], cxB[0:64, :],
                                         rbs1[:])

                # Dummy matmuls at the head of the PE queue: keep the PE
                # array busy while the first weight DMAs land so the HAM
                # clock gate opens (K=8) before the real chains start.
                warm = sc.tile([P, 512], f16, tag="warm", name="warm")
                nc.vector.memset(warm[:], 0.0)
                for i in range(40):
                    wp = pp.tile([P, 512], f32, tag="ps", name="wp")
                    nc.tensor.matmul(wp[:], warm[:, 0:P], warm[:],
                                     start=True, stop=True)

                # Emission order tracks DMA arrival: Q chains first (xqT +
                # wq first half), then V tiles 0-3 (xT first half + wv),
                # then K staged by xT halves, then the t=2..7 groups with
                # attention pair t-1 interleaved after each. The full
                # out-projection is the tail: its matmuls keep the PE
                # dense while attention pair 7's exp drains on ScalarE.
                for t in range(4):
                    emit_q(t)
                for t in range(4):
                    emit_v(t)
                emit_k(0, 0)
                emit_k(1, 0)
                for t in range(4, NT):
                    emit_v(t)
                emit_k(0, 1)
                emit_k(1, 1)
                emit_attn(0)
                for t in range(2, NT):
                    if t >= 4:
                        emit_q(t)
                    emit_k(t, 0)
                    emit_k(t, 1)
                    emit_attn(t - 1)
                while pending:
                    flush_one()
                emit_attn(NT - 1)

                # ---- output projection (alternating pp/scp banks) ----
                for idx, (i, n) in enumerate(
                        (i, n) for i in range(4) for n in range(2)):
                    pool, tag = (pp, "ps") if idx % 2 == 0 else (scp, "s")
                    yp = pool.tile([P, 512], f32, tag=tag,
                                   name=f"yp{i}_{n}")
                    csl = slice(512 * n, 512 * (n + 1))
                    for t in range(NT):
                        nc.tensor.matmul(yp[:], cn[:, t, P * i:P * (i + 1)],
                                         w_sb["wo"][:, t, csl],
                                         start=(t == 0),
                                         stop=(t == NT - 1))
                    ys = sc.tile([P, 512], f16, tag="ys", name="ys")
                    nc.vector.tensor_add(ys[:], yp[:], bob[:, csl])
                    nc.sync.dma_start(y_sh[P * i:P * (i + 1), csl], ys[:])

            if taps:
                for tn, tile_ap in (("qt", qt), ("kt", kt), ("v1", v1),
                                    ("cn", cn)):
                    nc.sync.dma_start(tap_ext[tn][:], tile_ap[:])

    nc.compile()
    return nc


def _host_tables():
    # RoPE tables, computed in float32 to match the reference's jnp path.
    pos = np.arange(S, dtype=np.float32)
    inv = np.exp(np.arange(0, Dh, 2, dtype=np.float32)
                 * np.float32(-np.log(10000.0) / Dh))          # [32]
    ang = pos[:, None] * inv[None, :]                          # [S, 32]
    sin = np.sin(ang).astype(np.float32)
    cos = np.cos(ang).astype(np.float32)
    # per-partition pattern for [2 heads x 64, s] transposed layout
    dd = np.arange(P) % Dh
    cosP = np.empty((P, S), np.float32)
    sinP = np.empty((P, S), np.float32)
    lo = dd < 32
    cosP[lo] = cos[:, dd[lo]].T
    sinP[lo] = -sin[:, dd[lo]].T
    cosP[~lo] = cos[:, dd[~lo] - 32].T
    sinP[~lo] = sin[:, dd[~lo] - 32].T
    return cosP.astype(np.float16), sinP.astype(np.float16)


def _perm128():
    p = np.zeros((P, P), np.float16)
    i = np.arange(P)
    p[i, i ^ 32] = np.float16(1.0)
    return p


def _tile_T(a):
    # [rows, D] -> [P, NT, rows]: partition-tiled transpose for SBUF layout
    rows = a.shape[0]
    return np.ascontiguousarray(a.T.reshape(NT, P, rows).transpose(1, 0, 2))


def make_in_maps(x, Wq, bq, Wk, bk, Wv, bv, Wo, bo):
    x = np.asarray(x, np.float16)
    shared = {
        "wq": np.ascontiguousarray(np.asarray(Wq, np.float16)),
        "wk": np.ascontiguousarray(np.asarray(Wk, np.float16)),
        "wv": np.ascontiguousarray(np.asarray(Wv, np.float16)),
        "wo": np.ascontiguousarray(np.asarray(Wo, np.float16)),
        "bqt": np.ascontiguousarray(
            np.asarray(bq, np.float16).astype(np.float32).reshape(NT, P).T),
        "bkt": np.ascontiguousarray(
            np.asarray(bk, np.float16).astype(np.float32).reshape(NT, P).T),
        "bvb": np.ascontiguousarray(np.broadcast_to(
            np.asarray(bv, np.float16).reshape(1, D), (P, D))),
        "bob": np.ascontiguousarray(np.broadcast_to(
            np.asarray(bo, np.float16).reshape(1, D), (P, D))),
        "p128": _perm128(),
    }
    cosP, sinP = _host_tables()
    shared["cosk"] = cosP
    shared["sink"] = sinP

    in_maps = []
    r = np.arange(P)[:, None]
    cc2 = 2 * np.arange(64)[None, :]
    for core in range(N_CORES):
        b, par = core // 2, core % 2
        xb = x[b]                                   # [1024, 1024]
        # plain q order: col c <-> seq 2c+par
        xq = xb[par::2]                             # [512, 1024]
        m = {
            "xt_sh": _tile_T(xb),
            "xqt_sh": _tile_T(xq),
            "cosq": np.ascontiguousarray(cosP[:, par::2]),
            "sinq": np.ascontiguousarray(sinP[:, par::2]),
            "mj": (r <= cc2 + par).astype(np.float16),
        }
        m.update(shared)
        in_maps.append(m)
    return in_maps


def kernel(x, Wq, bq, Wk, bk, Wv, bv, Wo, bo):
    from concourse.bass_utils import run_bass_kernel_spmd

    with _lock:
        if "nc" not in _cache:
            _cache["nc"] = _build_program()
    nc = _cache["nc"]

    in_maps = make_in_maps(x, Wq, bq, Wk, bk, Wv, bv, Wo, bo)
    res = run_bass_kernel_spmd(nc, in_maps, list(range(N_CORES)))

    out = np.empty((B, S, D), np.float16)
    o2 = out.reshape(B, 512, 2, D)
    for core in range(N_CORES):
        b, par = core // 2, core % 2
        o2[b, :, par, :] = res.results[core]["y_sh"]
    return out



# revision 36
# speedup vs baseline: 1.2420x; 1.0035x over previous
"""Trainium2 Bass kernel for CustomMultiHeadAttention (B=4, S=1024, D=1024, H=16, Dh=64).

Sharding: 8 cores = (batch b in 0..3) x (parity par in 0..1).
Core (b, par) computes output rows {s : s % 2 == par} of batch b,
grouped into 4 "vblocks" of 128 rows (vblock i' = seq 256*i' + 2*c + par).
K/V are computed for the full sequence on every core (from the full x[b]).
The program is identical on all cores; per-core differences are input data.

Pipeline (all transposed-layout, PE-centric):
  QT = rope(Wq^T x^T), KT = rope(Wk^T x^T)  - rope via permutation-matmul + DVE
  scT[kv,q] = KT_h^T QT_h (2 heads row-packed), exp on ScalarE (scale=1/8),
  causal mask = f16 0/1 multiply on the diagonal 128 cols,
  ctxT/denoms accumulate via lhsT=[V|1], normalize via reciprocal_approx_fast
  + PE broadcast, out = ctxT^T Wo.
"""

import threading

import numpy as np

B, S, D, H, Dh = 4, 1024, 1024, 16, 64
P = 128
N_CORES = 8
NT = D // P  # 8 tiles along d/dout/seq
# q col c <-> seq 2c+par (plain order); for kv-block j the active q
# cols are the suffix [64j, 512) and the mask covers its first 64 cols
NJ = [512 - 64 * j for j in range(8)]
VS = 65  # V slot width: [V(64) | ones(1)] per head

_cache = {}
_lock = threading.Lock()


def _build_program(taps=False):
    import concourse.bass as bass  # noqa: F401
    import concourse.mybir as mybir
    import concourse.tile as tile
    from concourse import bacc

    dt = mybir.dt
    f16, f32 = dt.float16, dt.float32
    AF = mybir.ActivationFunctionType

    nc = bacc.Bacc("TRN2", target_bir_lowering=False, debug=False,
                   num_devices=N_CORES)

    def ein(name, shape):
        return nc.dram_tensor(name, shape, f16, kind="ExternalInput").ap()

    xt_sh = ein("xt_sh", [P, NT, S])     # x[b]^T, host-transposed
    xqt_sh = ein("xqt_sh", [P, NT, 512])  # xq^T, host-transposed
    w_ext = {n: ein(n, [D, D]) for n in ("wq", "wk", "wv", "wo")}
    bqt_e = nc.dram_tensor("bqt", [P, NT], f32, kind="ExternalInput").ap()
    bkt_e = nc.dram_tensor("bkt", [P, NT], f32, kind="ExternalInput").ap()
    bvb_e = ein("bvb", [P, D])           # bv broadcast across partitions
    bob_e = ein("bob", [P, D])           # bo broadcast across partitions
    cosq_e = ein("cosq", [P, 512])
    sinq_e = ein("sinq", [P, 512])
    cosk_e = ein("cosk", [P, S])
    sink_e = ein("sink", [P, S])
    mj_e = ein("mj", [P, 64])
    p128_e = ein("p128", [P, P])
    y_sh = nc.dram_tensor("y_sh", [512, D], f16, kind="ExternalOutput").ap()
    tap_ext = {}
    if taps:
        for tn, shape in (("qt", [P, NT, 512]), ("kt", [P, NT, S]),
                          ("v1", [P, NT, H * VS]), ("cn", [P, NT, 512])):
            tap_ext[tn] = nc.dram_tensor("dbg_" + tn, shape, f16,
                                         kind="ExternalOutput").ap()

    with tile.TileContext(nc) as tc:
        from contextlib import ExitStack
        with ExitStack() as ctx:
            big = ctx.enter_context(tc.tile_pool(name="big", bufs=1))

            xT = big.tile([P, NT, S], f16, tag="xT")        # x[b]^T  [din, s]
            xqT = big.tile([P, NT, 512], f16, tag="xqT")    # xq^T    [din, q]
            w_sb = {n: big.tile([P, NT, D], f16, tag=n, name=n + "_sb")
                    for n in w_ext}
            bqt = big.tile([P, NT], f32, tag="bqt")
            bkt = big.tile([P, NT], f32, tag="bkt")
            bvb = big.tile([P, D], f16, tag="bvb")
            bob = big.tile([P, D], f16, tag="bob")
            qt = big.tile([P, NT, 512], f16, tag="qt")      # rope'd Q^T
            kt = big.tile([P, NT, S], f16, tag="kt")        # rope'd K^T
            v1 = big.tile([P, NT, H * VS], f16, tag="v1")   # [V|1] slots
            cn = big.tile([P, NT, 512], f16, tag="cn")      # normalized ctx^T
            cosq = big.tile([P, 512], f16, tag="cosq")
            sinq = big.tile([P, 512], f16, tag="sinq")
            cosk = big.tile([P, S], f16, tag="cosk")
            sink = big.tile([P, S], f16, tag="sink")
            mj = big.tile([P, 64], f16, tag="mj")
            p128 = big.tile([P, P], f16, tag="p128")

            # ---- input DMAs ----
            # Coarse DMAs: one dma_start per tensor chunk (issue overhead
            # is ~0.6us each, so fewer+bigger wins). Order on the sync
            # queue matches first-use order of the projection chains.
            def wslice(name, c0, c1):
                src = w_ext[name].rearrange("(k p) d -> p k d", p=P)
                nc.sync.dma_start(w_sb[name][:, :, c0:c1],
                                  src[:, :, c0:c1])

            # x and the small const tables go on the scalar queue (idle
            # early) so the sync queue streams only the big weight slices
            nc.scalar.dma_start(xqT[:], xqt_sh[:])
            for t, e in ((p128, p128_e), (cosq, cosq_e),
                         (sinq, sinq_e), (bqt, bqt_e),
                         (cosk, cosk_e), (sink, sink_e), (bkt, bkt_e)):
                nc.scalar.dma_start(t[:], e[:])
            wslice("wq", 0, 512)
            nc.sync.dma_start(xT[:, :, 0:512], xt_sh[:, :, 0:512])
            wslice("wk", 0, 512)
            nc.sync.dma_start(xT[:, :, 512:1024], xt_sh[:, :, 512:1024])
            wslice("wq", 512, 1024)
            wslice("wk", 512, 1024)
            # bulk weights on the gpsimd queue, in parallel
            nc.gpsimd.dma_start(
                w_sb["wv"][:],
                w_ext["wv"].rearrange("(k p) d -> p k d", p=P))
            for t, e in ((bvb, bvb_e), (mj, mj_e)):
                nc.gpsimd.dma_start(t[:], e[:])
            nc.gpsimd.dma_start(
                w_sb["wo"][:],
                w_ext["wo"].rearrange("(k p) d -> p k d", p=P))
            nc.gpsimd.dma_start(bob[:], bob_e[:])
            # ones columns of the V slots (col 64 of each 65-wide slot)
            v1r = v1.rearrange("p t (h c) -> p t h c", c=VS)
            for t in range(NT):
                nc.any.memset(v1r[:, t, :, 64:65], 1.0)

            # ---- projections + rope + attention, interleaved ----
            # Projections use a single rotating PSUM tag (3 banks); the
            # perm-matmul for rope unit u-1 is emitted after unit u's chain
            # so the PE never waits on the ScalarE psum->sbuf evacuation.
            # Attention head-pair p is emitted right after the t=p+1
            # projections, so its ScalarE exp work overlaps the remaining
            # projection matmuls instead of serializing behind them.
            # PSUM banks: pp 3 + scp 3 + cxp 2 = 8.
            with tc.tile_pool(name="pp", bufs=3, space="PSUM") as pp, \
                 tc.tile_pool(name="sc", bufs=4) as sc, \
                 tc.tile_pool(name="scp", bufs=3, space="PSUM") as scp, \
                 tc.tile_pool(name="cxp", bufs=1, space="PSUM") as cxp, \
                 tc.tile_pool(name="ep", bufs=3) as ep, \
                 tc.tile_pool(name="npl", bufs=2) as npl:
                pending = []

                def flush_one():
                    dst, raw, cos_ap, sin_ap = pending.pop(0)
                    pq = pp.tile([P, 512], f32, tag="ps", name="pq")
                    nc.tensor.matmul(pq[:], p128[:], raw[:],
                                     start=True, stop=True)
                    t1 = sc.tile([P, 512], f16, tag="t1", name="t1")
                    nc.vector.tensor_mul(t1[:], raw[:], cos_ap)
                    t2 = sc.tile([P, 512], f16, tag="t2", name="t2")
                    nc.vector.tensor_mul(t2[:], pq[:], sin_ap)
                    nc.vector.tensor_add(dst, t1[:], t2[:])

                def rope_chain(dst, w_name, bias_col, rhs, cos_ap, sin_ap,
                               dst_sl):
                    ps = pp.tile([P, 512], f32, tag="ps", name="ps")
                    for k in range(NT):
                        nc.tensor.matmul(ps[:], w_sb[w_name][:, k, dst_sl],
                                         rhs(k), start=(k == 0),
                                         stop=(k == NT - 1))
                    # psum->sbuf f16 with fused per-partition bias (ScalarE)
                    raw = sc.tile([P, 512], f16, tag="raw", name="raw")
                    nc.scalar.activation(raw[:], ps[:], AF.Identity,
                                         bias=bias_col)
                    pending.append((dst, raw, cos_ap, sin_ap))
                    if len(pending) > 1:
                        flush_one()

                def emit_q(t):
                    dst_sl = slice(P * t, P * (t + 1))
                    rope_chain(qt[:, t, :], "wq", bqt[:, t:t + 1],
                               lambda k: xqT[:, k, :], cosq[:], sinq[:],
                               dst_sl)

                def emit_k(t, n):
                    dst_sl = slice(P * t, P * (t + 1))
                    csl = slice(512 * n, 512 * (n + 1))
                    rope_chain(kt[:, t, csl], "wk", bkt[:, t:t + 1],
                               lambda k, csl=csl: xT[:, k, csl],
                               cosk[:, csl], sink[:, csl], dst_sl)

                def emit_v(t):
                    # V tile t (s-tile): natural [s, dout] into 65-wide slots
                    dst_sl = slice(P * t, P * (t + 1))
                    for n in range(2):
                        csl = slice(512 * n, 512 * (n + 1))
                        vp = pp.tile([P, 512], f32, tag="ps", name="vp")
                        for k in range(NT):
                            nc.tensor.matmul(vp[:], xT[:, k, dst_sl],
                                             w_sb["wv"][:, k, csl],
                                             start=(k == 0),
                                             stop=(k == NT - 1))
                        nc.vector.tensor_add(
                            v1r[:, t, 8 * n:8 * n + 8, 0:64],
                            vp.rearrange("p (h c) -> p h c", c=64),
                            bvb.rearrange("p (n h c) -> p n h c",
                                          n=2, c=64)[:, n])

                def emit_attn(p):
                    # attention head pair p (heads 2p, 2p+1); ctx+denom
                    # fused: stationary is the 65-wide [V|1] slot, PSUM
                    # rows 0:64 accumulate ctx and row 64 the denominator
                    cxA = cxp.tile([VS, 512], f32, tag="cxA", name="cxA")
                    cxB = cxp.tile([VS, 512], f32, tag="cxB", name="cxB")
                    h0, h1 = 2 * p, 2 * p + 1
                    es = {}

                    def emit_scores(j):
                        N = NJ[j]
                        co = 512 - N
                        e = ep.tile([P, 1024], f16, tag="e",
                                    name=f"e{p}_{j}")
                        for h in range(2):
                            rsl = slice(64 * h, 64 * (h + 1))
                            s_ps = scp.tile([P, 512], f32, tag="s",
                                            name=f"s{p}_{j}_{h}")
                            nc.tensor.matmul(s_ps[:, 0:N],
                                             kt[rsl, p, P * j:P * (j + 1)],
                                             qt[rsl, p, co:512],
                                             start=True, stop=True,
                                             skip_group_check=True)
                            nc.scalar.activation(e[:, 512 * h:512 * h + N],
                                                 s_ps[:, 0:N],
                                                 AF.Exp, scale=0.125)
                            nc.vector.tensor_mul(
                                e[:, 512 * h:512 * h + 64],
                                e[:, 512 * h:512 * h + 64], mj[:])
                        es[j] = e

                    def emit_ctx(j):
                        N = NJ[j]
                        co = 512 - N
                        e = es.pop(j)
                        st, sp = (j == 0), (j == NT - 1)
                        nc.tensor.matmul(cxA[:, co:512],
                                         v1[:, j, VS * h0:VS * h0 + VS],
                                         e[:, 0:N], start=st, stop=sp)
                        nc.tensor.matmul(cxB[:, co:512],
                                         v1[:, j, VS * h1:VS * h1 + VS],
                                         e[:, 512:512 + N],
                                         start=st, stop=sp)

                    # depth-2 software pipeline: scores run ahead of ctx
                    for j in range(NT + 2):
                        if j < NT:
                            emit_scores(j)
                        if j >= 2:
                            emit_ctx(j - 2)

                    # normalize: denom rows -> one [1,1024] copy pair, one
                    # recip, gpsimd broadcast, per-head multiply
                    dd = npl.tile([1, 1024], f32, tag="d", name="dd")
                    nc.vector.tensor_copy(dd[:, 0:512], cxA[64:65, :])
                    nc.vector.tensor_copy(dd[:, 512:1024], cxB[64:65, :])
                    rr = npl.tile([1, 1024], f32, tag="r", name="rr")
                    nc.vector.reciprocal_approx_fast(rr[:], dd[:])
                    rbs0 = npl.tile([64, 512], f32, tag="rb", name="rbs0")
                    nc.gpsimd.partition_broadcast(rbs0[:], rr[:, 0:512],
                                                  channels=64)
                    rbs1 = npl.tile([64, 512], f32, tag="rb", name="rbs1")
                    nc.gpsimd.partition_broadcast(rbs1[:], rr[:, 512:1024],
                                                  channels=64)
                    nc.vector.tensor_mul(cn[0:64, p, :], cxA[0:64, :],
                                         rbs0[:])
                    nc.vector.tensor_mul(cn[64:P, p, :], cxB[0:64, :],
                                         rbs1[:])

                # Dummy matmuls at the head of the PE queue: keep the PE
                # array busy while the first weight DMAs land so the HAM
                # clock gate opens (K=8) before the real chains start.
                warm = sc.tile([P, 512], f16, tag="warm", name="warm")
                nc.vector.memset(warm[:], 0.0)
                for i in range(28):
                    wp = pp.tile([P, 512], f32, tag="ps", name="wp")
                    nc.tensor.matmul(wp[:], warm[:, 0:P], warm[:],
                                     start=True, stop=True)

                # Emission order tracks DMA arrival: Q chains first (xqT +
                # wq first half), then V tiles 0-3 (xT first half + wv),
                # then K staged by xT halves, then the t=2..7 groups with
                # attention pair t-1 interleaved after each. The full
                # out-projection is the tail: its matmuls keep the PE
                # dense while attention pair 7's exp drains on ScalarE.
                for t in range(4):
                    emit_q(t)
                for t in range(4):
                    emit_v(t)
                emit_k(0, 0)
                emit_k(1, 0)
                for t in range(4, NT):
                    emit_v(t)
                emit_k(0, 1)
                emit_k(1, 1)
                emit_attn(0)
                for t in range(2, NT):
                    if t >= 4:
                        emit_q(t)
                    emit_k(t, 0)
                    emit_k(t, 1)
                    emit_attn(t - 1)
                while pending:
                    flush_one()
                emit_attn(NT - 1)

                # ---- output projection (alternating pp/scp banks) ----
                for idx, (i, n) in enumerate(
                        (i, n) for i in range(4) for n in range(2)):
                    pool, tag = (pp, "ps") if idx % 2 == 0 else (scp, "s")
                    yp = pool.tile([P, 512], f32, tag=tag,
                                   name=f"yp{i}_{n}")
                    csl = slice(512 * n, 512 * (n + 1))
                    for t in range(NT):
                        nc.tensor.matmul(yp[:], cn[:, t, P * i:P * (i + 1)],
                                         w_sb["wo"][:, t, csl],
                                         start=(t == 0),
                                         stop=(t == NT - 1))
                    ys = sc.tile([P, 512], f16, tag="ys", name="ys")
                    nc.vector.tensor_add(ys[:], yp[:], bob[:, csl])
                    nc.sync.dma_start(y_sh[P * i:P * (i + 1), csl], ys[:])

            if taps:
                for tn, tile_ap in (("qt", qt), ("kt", kt), ("v1", v1),
                                    ("cn", cn)):
                    nc.sync.dma_start(tap_ext[tn][:], tile_ap[:])

    nc.compile()
    return nc


def _host_tables():
    # RoPE tables, computed in float32 to match the reference's jnp path.
    pos = np.arange(S, dtype=np.float32)
    inv = np.exp(np.arange(0, Dh, 2, dtype=np.float32)
                 * np.float32(-np.log(10000.0) / Dh))          # [32]
    ang = pos[:, None] * inv[None, :]                          # [S, 32]
    sin = np.sin(ang).astype(np.float32)
    cos = np.cos(ang).astype(np.float32)
    # per-partition pattern for [2 heads x 64, s] transposed layout
    dd = np.arange(P) % Dh
    cosP = np.empty((P, S), np.float32)
    sinP = np.empty((P, S), np.float32)
    lo = dd < 32
    cosP[lo] = cos[:, dd[lo]].T
    sinP[lo] = -sin[:, dd[lo]].T
    cosP[~lo] = cos[:, dd[~lo] - 32].T
    sinP[~lo] = sin[:, dd[~lo] - 32].T
    return cosP.astype(np.float16), sinP.astype(np.float16)


def _perm128():
    p = np.zeros((P, P), np.float16)
    i = np.arange(P)
    p[i, i ^ 32] = np.float16(1.0)
    return p


def _tile_T(a):
    # [rows, D] -> [P, NT, rows]: partition-tiled transpose for SBUF layout
    rows = a.shape[0]
    return np.ascontiguousarray(a.T.reshape(NT, P, rows).transpose(1, 0, 2))


def make_in_maps(x, Wq, bq, Wk, bk, Wv, bv, Wo, bo):
    x = np.asarray(x, np.float16)
    shared = {
        "wq": np.ascontiguousarray(np.asarray(Wq, np.float16)),
        "wk": np.ascontiguousarray(np.asarray(Wk, np.float16)),
        "wv": np.ascontiguousarray(np.asarray(Wv, np.float16)),
        "wo": np.ascontiguousarray(np.asarray(Wo, np.float16)),
        "bqt": np.ascontiguousarray(
            np.asarray(bq, np.float16).astype(np.float32).reshape(NT, P).T),
        "bkt": np.ascontiguousarray(
            np.asarray(bk, np.float16).astype(np.float32).reshape(NT, P).T),
        "bvb": np.ascontiguousarray(np.broadcast_to(
            np.asarray(bv, np.float16).reshape(1, D), (P, D))),
        "bob": np.ascontiguousarray(np.broadcast_to(
            np.asarray(bo, np.float16).reshape(1, D), (P, D))),
        "p128": _perm128(),
    }
    cosP, sinP = _host_tables()
    shared["cosk"] = cosP
    shared["sink"] = sinP

    in_maps = []
    r = np.arange(P)[:, None]
    cc2 = 2 * np.arange(64)[None, :]
    for core in range(N_CORES):
        b, par = core // 2, core % 2
        xb = x[b]                                   # [1024, 1024]
        # plain q order: col c <-> seq 2c+par
        xq = xb[par::2]                             # [512, 1024]
        m = {
            "xt_sh": _tile_T(xb),
            "xqt_sh": _tile_T(xq),
            "cosq": np.ascontiguousarray(cosP[:, par::2]),
            "sinq": np.ascontiguousarray(sinP[:, par::2]),
            "mj": (r <= cc2 + par).astype(np.float16),
        }
        m.update(shared)
        in_maps.append(m)
    return in_maps


def kernel(x, Wq, bq, Wk, bk, Wv, bv, Wo, bo):
    from concourse.bass_utils import run_bass_kernel_spmd

    with _lock:
        if "nc" not in _cache:
            _cache["nc"] = _build_program()
    nc = _cache["nc"]

    in_maps = make_in_maps(x, Wq, bq, Wk, bk, Wv, bv, Wo, bo)
    res = run_bass_kernel_spmd(nc, in_maps, list(range(N_CORES)))

    out = np.empty((B, S, D), np.float16)
    o2 = out.reshape(B, 512, 2, D)
    for core in range(N_CORES):
        b, par = core // 2, core % 2
        o2[b, :, par, :] = res.results[core]["y_sh"]
    return out

